# revision 15
# baseline (speedup 1.0000x reference)
"""Linear-attention (ELU+1 feature map) Bass kernel for TRN2, 8 NeuronCores.

Problem: B=8, N=4096, C=512, 8 heads, d=64.
  q = x @ Wq.T;  kv = x @ Wkv.T -> k, v
  Q = elu(q)+1; K = elu(k)+1
  KV[h,d,v] = sum_s K[s,h,d] v[s,h,v]
  den[l,h]  = Q[l,h,:] . sum_s K[s,h,:]  (+eps, negligible vs den~1e5)
  out[l,h,v] = (sum_d Q[l,h,d] KV[h,d,v]) / den[l,h]
  (the reference's /N on v and *N on out cancel)

Sharding: data-parallel over B - core b computes batch b. No collectives.

Host prep (inside kernel()): cast x/Wq/Wkv to bf16 and pre-transpose so the
device receives xT [C, N], WqT [C, C], WkvT [C, 2C] - this removes all PE
transposes and their PSUM round-trips. Output is written bf16 and upcast on
host.

Single-core dataflow, all matmuls bf16 (fp32 PSUM accum):
  phase 1 (per 512-token macro-tile, xT chunks DMA'd directly):
    - qT[o,tok] = WqT.T @ xT  -> elu+1 -> QT (bf16, resident)
    - k,v[tok,o] = xT.T @ WkvT -> elu+1 -> K (bf16); v -> V_aug (ones col)
    - KV_aug += K_chunk.T @ V_aug_chunk accumulates KV+Ksum in PSUM.
  elu+1 = (exp(x) min 1) + relu(x): exp ACT; relu DVE (q) / ACT (k);
    combine: fused stt DVE (q) / min DVE + add GPSIMD (k).
  phase 2 (per 128-token tile):
    - [num | den] = QT_chunk.T @ kvdw_chunk   (N=130)
    - zr = 1/den (DVE reciprocal); out = num * zr on one of three lanes
      (DVE mult / ACT scaled copies / ACT copy + GPSIMD mult), hand-tuned
      40:16:8 against HW (ALU divide is rejected by the BIR verifier,
      GPSIMD cannot read PSUM and only has add/mult/copy/memset).
"""
import contextlib
import os
import sys

for _p in ("/opt/trn_rl_repo", "/root/.axon_site/_ro/trn_rl_repo"):
    if os.path.isdir(_p) and _p not in sys.path:
        sys.path.insert(0, _p)

import numpy as np

import concourse.bass as bass
import concourse.tile as tile
from concourse import bacc, mybir
from concourse.bass_utils import run_bass_kernel_spmd

dt = mybir.dt
AF = mybir.ActivationFunctionType
ALU = mybir.AluOpType

N_CORES = 8
B, N, C = 8, 4096, 512
H, D = 8, 64
P = 128          # partitions / tile row count
CH = C // P      # 4 contraction chunks
NT = N // P      # 32 token tiles
TM = 4           # token tiles per macro-tile
NM = NT // TM    # 8 macro-tiles
W_AUG = P + 1    # 129: per-chunk KV columns incl. ones column
W2 = P + 2       # 130: phase-2 rhs cols (128 num + 2 den)
OT = 4           # output tiles per DMA


def _elu1(nc, pool, out_ap, src_psum, relu_on_act):
    """out = elu(src)+1 = (exp(src) min 1) + relu(src), from PSUM.

    exp is ACT-only; the fused min+add stt is DVE-only (TensorScalarPtr
    is not a legal Pool opcode).  The relu engine is the balance knob:
    ACT has ~530 ns fixed cost per op on HW, so only 2 of the 8 relus
    per macro-tile go there (ACT ~10.5 us/macro vs PE 11.1).
    """
    p, f = src_psum.shape[0], src_psum.shape[1]
    e = pool.tile([p, f], dt.bfloat16, name="elu_e", tag="elu_e", bufs=4)
    nc.scalar.activation(e[:], src_psum, AF.Exp)
    r = pool.tile([p, f], dt.bfloat16, name="elu_r", tag="elu_r", bufs=4)
    if relu_on_act:
        nc.scalar.activation(r[:], src_psum, AF.Relu)
    else:
        nc.vector.tensor_scalar_max(r[:], src_psum, 0.0)
    nc.vector.scalar_tensor_tensor(
        out_ap, e[:], 1.0, r[:], op0=ALU.min, op1=ALU.add
    )


def build_nc(loop_reps=1, ablate=()):
    nc = bacc.Bacc("TRN2", target_bir_lowering=False, debug=False,
                   num_devices=N_CORES)
    xT_ext = nc.dram_tensor("xT", (C, N), dt.bfloat16, kind="ExternalInput")
    wqT_ext = nc.dram_tensor("WqT", (C, C), dt.bfloat16, kind="ExternalInput")
    wkvT_ext = nc.dram_tensor("WkvT", (C, 2 * C), dt.bfloat16,
                              kind="ExternalInput")
    out_ext = nc.dram_tensor("out", (N, C), dt.bfloat16, kind="ExternalOutput")

    with tile.TileContext(nc) as tc:
        with tc.tile_pool(name="sb_w", bufs=1) as sb_w, \
             tc.tile_pool(name="sb_qt", bufs=1) as sb_qt, \
             tc.tile_pool(name="sb", bufs=1) as sb, \
             tc.tile_pool(name="ps", bufs=1, space="PSUM") as ps, \
             tc.tile_pool(name="ps_acc", bufs=1, space="PSUM") as ps_acc:

            rep_ctx = (tc.For_i(0, loop_reps, 1) if loop_reps > 1
                       else contextlib.nullcontext())
            with rep_ctx:
                _build_body(nc, tc, sb_w, sb_qt, sb, ps, ps_acc,
                            xT_ext, wqT_ext, wkvT_ext, out_ext, ablate)

    nc.compile()
    return nc


def _build_body(nc, tc, sb_w, sb_qt, sb, ps, ps_acc,
                xT_ext, wqT_ext, wkvT_ext, out_ext, ablate=()):
    # ---------------- weights (already transposed on host) ----------------
    # Weight DMAs go on the Pool (SWDGE) queue so they don't serialize
    # behind the first x DMA on SP; q weights first (q matmuls come first).
    wqT = sb_w.tile([P, CH, C], dt.bfloat16, name="wqT")
    nc.gpsimd.dma_start(wqT[:], wqT_ext[:].rearrange("(a p) o -> p a o", p=P))
    wkvT = sb_w.tile([P, CH, 2 * C], dt.bfloat16, name="wkvT")
    nc.gpsimd.dma_start(wkvT[:], wkvT_ext[:].rearrange("(a p) o -> p a o",
                                                       p=P))

    # resident Q^T, bf16: 4 chunks [128, 4096]
    qT = [sb_qt.tile([P, N], dt.bfloat16, name=f"qT{ci}")
          for ci in range(CH)]
    # persistent KV accumulation PSUM: 2 banks, 2 chunks per bank.
    # start=True clears has_written for the WHOLE bank, so clear each bank
    # once with a K=1 zero matmul; all accumulating matmuls use start=False.
    kv_ps = ps_acc.tile([P, 2, 512], dt.float32, name="kv_ps")
    zlhs = sb_w.tile([1, P], dt.bfloat16, name="zlhs")
    zrhs = sb_w.tile([1, 512], dt.bfloat16, name="zrhs")
    nc.vector.memset(zlhs[:], 0.0)
    nc.vector.memset(zrhs[:], 0.0)
    for bk in range(2):
        nc.tensor.matmul(kv_ps[:, bk, :], zlhs[:], zrhs[:],
                         start=True, stop=True)

    # ---------------- phase 1 ----------------
    for mi in range(NM):
        t0 = mi * TM * P
        xTc = sb.tile([P, CH, TM * P], dt.bfloat16, name="xT",
                      tag="xT", bufs=3)
        if mi == 0:
            # per-chunk DMAs so the first q matmul starts after chunk 0
            # lands instead of after the whole macro-tile
            for ci in range(CH):
                nc.sync.dma_start(
                    xTc[:, ci, :],
                    xT_ext[ci * P:(ci + 1) * P, t0:t0 + TM * P])
        else:
            nc.sync.dma_start(
                xTc[:], xT_ext[:, t0:t0 + TM * P].rearrange(
                    "(a p) t -> p a t", p=P))

        # q^T chunks: [o 128, 512 tok]
        for oj in () if "proj" in ablate else range(CH):
            pq = ps.tile([P, TM * P], dt.float32, name="pq",
                         tag="pq", bufs=3)
            for ci in range(CH):
                nc.tensor.matmul(
                    pq[:], wqT[:, ci, oj * P:(oj + 1) * P], xTc[:, ci, :],
                    start=(ci == 0), stop=(ci == CH - 1),
                )
            _elu1(nc, sb, qT[oj][:, t0:t0 + TM * P], pq[:], False)

        # k, v (token-major) + KV accumulation
        for tj in () if "proj" in ablate else range(TM):
            pk = ps.tile([P, C], dt.float32, name="pk", tag="pkv", bufs=3)
            pv = ps.tile([P, C], dt.float32, name="pv", tag="pkv", bufs=3)
            for ci in range(CH):
                nc.tensor.matmul(
                    pk[:], xTc[:, ci, tj * P:(tj + 1) * P],
                    wkvT[:, ci, 0:C],
                    start=(ci == 0), stop=(ci == CH - 1),
                )
            for ci in range(CH):
                nc.tensor.matmul(
                    pv[:], xTc[:, ci, tj * P:(tj + 1) * P],
                    wkvT[:, ci, C:2 * C],
                    start=(ci == 0), stop=(ci == CH - 1),
                )
            ksb = sb.tile([P, C], dt.bfloat16, name="ksb",
                          tag="ksb", bufs=3)
            _elu1(nc, sb, ksb[:], pk[:], relu_on_act=(tj % 2 == 0))
            vaug = sb.tile([P, CH * W_AUG], dt.bfloat16, name="vaug",
                           tag="vaug", bufs=3)
            vv = vaug[:].rearrange("p (c w) -> p c w", w=W_AUG)
            # GPSIMD cannot read PSUM; v copy 3x DVE / 1x ACT
            if tj != 3:
                nc.vector.tensor_copy(
                    vv[:, :, 0:P], pv[:].rearrange("p (c w) -> p c w", w=P))
            else:
                nc.scalar.activation(
                    vv[:, :, 0:P], pv[:].rearrange("p (c w) -> p c w", w=P),
                    AF.Copy)
            nc.gpsimd.memset(vv[:, :, P:W_AUG], 1.0)

            last = (mi == NM - 1 and tj == TM - 1)
            if "kv" not in ablate:
                for c in range(CH):
                    nc.tensor.matmul(
                        kv_ps[:, c // 2,
                              (c % 2) * W_AUG:(c % 2 + 1) * W_AUG],
                        ksb[:, c * P:(c + 1) * P],
                        vaug[:, c * W_AUG:(c + 1) * W_AUG],
                        start=False, stop=last,
                        skip_group_check=True,
                    )

    # ---------------- phase boundary ----------------
    # kvdw bf16 [128, 4*130]: per chunk [KV diag blocks (128) | ksum 2 cols]
    kvdw = sb_w.tile([P, CH * W2], dt.bfloat16, name="kvdw")
    nc.gpsimd.memset(kvdw[:], 0.0)
    for c in range(CH):
        bk, co = c // 2, (c % 2) * W_AUG
        o2 = c * W2
        # alternate copies across DVE / ACT to shorten the boundary
        nc.vector.tensor_copy(
            kvdw[0:D, o2:o2 + D], kv_ps[0:D, bk, co:co + D])
        nc.scalar.activation(
            kvdw[D:P, o2 + D:o2 + P], kv_ps[D:P, bk, co + D:co + P],
            AF.Copy)
        nc.vector.tensor_copy(
            kvdw[0:D, o2 + P:o2 + P + 1],
            kv_ps[0:D, bk, co + P:co + W_AUG])
        nc.vector.tensor_copy(
            kvdw[D:P, o2 + P + 1:o2 + W2],
            kv_ps[D:P, bk, co + P:co + W_AUG])

    # ---------------- phase 2 ----------------
    if "ph2" in ablate:
        dummy = sb.tile([P, C], dt.bfloat16, name="dummy_o", tag="osb",
                        bufs=2)
        nc.vector.memset(dummy[:], 0.0)
        nc.sync.dma_start(out_ext[0:P, :], dummy[:])
        return
    # Normalize+store: the BIR verifier rejects ALU divide on every engine
    # and GPSIMD cannot read PSUM, so each half-tile takes a DVE reciprocal
    # of its den columns plus a zr multiply on one of three lanes:
    #   D: DVE tensor_tensor mult from PSUM        (~390 ns DVE)
    #   A: 4 per-head ACT copies with scale=zr     (~1000 ns ACT)
    #   C: ACT plain copy PSUM->SBUF bf16 (~420 ns ACT)
    #      + GPSIMD SBUF mult with zr broadcast    (~650 ns Pool)
    counts = {"D": 40, "C": 16, "A": 8}
    credit = {k: 0.0 for k in counts}
    lanes = []
    for _ in range(64):
        for k in counts:
            credit[k] += counts[k] / 64
        pick = max(credit, key=credit.get)
        credit[pick] -= 1.0
        lanes.append(pick)
    half_idx = 0
    om_sizes = [OT] * (NT // OT)
    om_starts = [i * OT for i in range(NT // OT)]
    om_of = {}
    for s, z in zip(om_starts, om_sizes):
        for t in range(s, s + z):
            om_of[t] = (s, z)
    for t in range(NT):
        gs, gz = om_of[t]
        if t == gs:
            om = sb.tile([P, gz, C], dt.bfloat16, name="om", tag="osb",
                         bufs=3)
        pnA = ps.tile([P, 2, W2], dt.float32, name="pnA", tag="pq", bufs=3)
        pnB = ps.tile([P, 2, W2], dt.float32, name="pnB", tag="pkv", bufs=3)
        for c in range(CH):
            pb = pnA if c < 2 else pnB
            nc.tensor.matmul(
                pb[:, c % 2, :],
                qT[c][:, t * P:(t + 1) * P],
                kvdw[:, c * W2:(c + 1) * W2],
                start=True, stop=True,
            )
        osb = om[:, t - gs]
        for b, pb in enumerate((pnA, pnB)):
            lane = lanes[half_idx % len(lanes)]
            half_idx += 1
            ob = osb[:, b * 256:(b + 1) * 256].rearrange(
                "p (c h w) -> p c h w", c=2, w=D)
            nm = pb[:, :, 0:P].rearrange("p c (h w) -> p c h w", w=D)
            den = pb[:, :, P:W2]
            zr = sb.tile([P, 2, 2], dt.float32, name="zr", tag="zr",
                         bufs=8)
            nc.vector.reciprocal(zr[:], den)
            if lane == "D":
                nc.vector.tensor_tensor(
                    ob, nm, zr[:].broadcast_to((P, 2, 2, D)), op=ALU.mult)
            elif lane == "C":
                tmpc = sb.tile([P, 2, 2, D], dt.bfloat16, name="tmpc",
                               tag="tmpc", bufs=6)
                nc.scalar.activation(tmpc[:], nm, AF.Copy)
                nc.gpsimd.tensor_tensor(
                    ob, tmpc[:], zr[:].broadcast_to((P, 2, 2, D)),
                    op=ALU.mult)
            else:
                for c in range(2):
                    for h in range(2):
                        nc.scalar.activation(
                            ob[:, c, h, :], nm[:, c, h, :], AF.Copy,
                            scale=zr[:, c, h:h + 1])
        if t == gs + gz - 1:
            r0 = gs * P
            nc.sync.dma_start(
                out_ext[r0:r0 + gz * P, :].rearrange(
                    "(a p) c -> p a c", p=P),
                om[:, 0:gz])


_NC_CACHE = None


def _get_nc():
    global _NC_CACHE
    if _NC_CACHE is None:
        _NC_CACHE = build_nc()
    return _NC_CACHE


def _prep(inputs):
    import ml_dtypes
    bf16 = ml_dtypes.bfloat16
    x = np.asarray(inputs["x"])
    wq = np.asarray(inputs["Wq"])
    wkv = np.asarray(inputs["Wkv"])
    xT = np.ascontiguousarray(
        x.astype(bf16).transpose(0, 2, 1))             # [B, C, N]
    wqT = np.ascontiguousarray(wq.astype(bf16).T)      # [C, C]
    wkvT = np.ascontiguousarray(wkv.astype(bf16).T)    # [C, 2C]
    return xT, wqT, wkvT


def run(inputs, trace=False, **kw):
    xT, wqT, wkvT = _prep(inputs)
    nc = _get_nc()
    in_maps = [{"xT": xT[b], "WqT": wqT, "WkvT": wkvT}
               for b in range(N_CORES)]
    res = run_bass_kernel_spmd(nc, in_maps, core_ids=list(range(N_CORES)),
                               trace=trace, **kw)
    out = np.stack([res.results[b]["out"].astype(np.float32)
                    for b in range(N_CORES)], axis=0)
    return out, res


def kernel(**inputs):
    out, _ = run(inputs)
    return out
